# revision 46
# baseline (speedup 1.0000x reference)
"""Trainium2 Bass kernel: e3nn-style GNN convolution (FC-net edge weights ->
FullyConnectedTensorProduct -> scatter-sum over edge_dst).

v3 strategy (edge-parallel, dst-sharded, host pre-gather):
  * Sort edges by dst on host. Core c owns dst nodes [2500c, 2500(c+1)).
  * Host precomputes per padded edge slot (20 blocks x t_b tiles x 128):
      eg  = [shs*s(16) | shs*v_ci(48) | dsc(16) | s(16) | shv(3) | pad] (100)
      Sg  = one-hot dst column (128, fp16)
      scT = edge scalars (3, for the on-device FC net)
  * Per 1024-edge group on device:
      - FC1 on PE per 512 edges (h^T in PSUM), relu on Act -> fp16
      - FC2 on PE per 128 edges: w[e,1024] in PSUM, Act copy -> fp16 SBUF
      - products: paths 1/2/4 on DVE (fp16 packed, 2x mode), path 3 and the
        final feature assembly on GpSimd
      - reduction over i as a pairwise tensor_tensor add tree on DVE
      - per 128-edge tile: one-hot dst matmul accumulates the block's
        [128,64] output in PSUM; block end -> Act copy -> DMA out.
  * Host concatenates the 8 node-sharded outputs.

All normalization constants are folded into fc_w1/fc_w2 on the host.
"""

import math

import numpy as np

N_NODES = 20000
N_CORES = 8
NODES_PER_CORE = N_NODES // N_CORES  # 2500
P = 128
MUL = 16
BLK = 128
BLOCKS = NODES_PER_CORE // BLK + 1  # 20 blocks cover 2560 rows
GG_E = 1024
OUT_ROWS = BLOCKS * BLK  # 2560
EGW = 100  # per-edge packed stream width

_CACHE: dict = {}


def _build(t_b: int, _unused: int = 0):
    import concourse.bass as bass
    import concourse.mybir as mybir
    import concourse.tile as tile
    from concourse import bacc

    dt = mybir.dt
    Alu = mybir.AluOpType
    Act = mybir.ActivationFunctionType

    n_tiles = BLOCKS * t_b
    EP = n_tiles * P
    assert EP % GG_E == 0
    ngg = EP // GG_E

    nc = bacc.Bacc("TRN2", target_bir_lowering=False, debug=False)

    egd = nc.dram_tensor("egd", [P, n_tiles * EGW], dt.float16, kind="ExternalInput")
    sgd = nc.dram_tensor("sgd", [P, n_tiles * P], dt.float16, kind="ExternalInput")
    scTd = nc.dram_tensor("scTd", [3, EP], dt.float16, kind="ExternalInput")
    fw1 = nc.dram_tensor("fw1", [3, 256], dt.float16, kind="ExternalInput")
    fw2d = nc.dram_tensor("fw2d", [P, 2048], dt.float16, kind="ExternalInput")
    outp = nc.dram_tensor("outp", [OUT_ROWS, 64], dt.float32, kind="ExternalOutput")

    with tile.TileContext(nc) as tc:
        with (
            tc.tile_pool(name="const", bufs=1) as cp,
            tc.tile_pool(name="egp", bufs=3) as egp,
            tc.tile_pool(name="sgp", bufs=3) as sgp,
            tc.tile_pool(name="fp", bufs=3) as fp,
            tc.tile_pool(name="hp", bufs=3) as hpool,
            tc.tile_pool(name="wp16", bufs=3) as wpool,
            tc.tile_pool(name="dv", bufs=2) as dv,
            tc.tile_pool(name="ob", bufs=2) as ob,
            tc.tile_pool(name="hps", bufs=1, space="PSUM") as hps,
            tc.tile_pool(name="wps", bufs=2, space="PSUM") as wps,
            tc.tile_pool(name="aps", bufs=1, space="PSUM") as aps,
            tc.tile_pool(name="aps4", bufs=1, space="PSUM") as aps4,
        ):
            scT_sb = cp.tile([3, EP], dt.float16)
            nc.sync.dma_start(scT_sb[:], scTd[:])
            fw1_sb = cp.tile([3, 256], dt.float16)
            nc.sync.dma_start(fw1_sb[:], fw1[:])
            fw2_sb = cp.tile([P, 2048], dt.float16)
            nc.sync.dma_start(fw2_sb[:], fw2d[:])

            acc = None
            for g in range(ngg):
                eg = egp.tile([P, 8, EGW], dt.float16, tag="eg")
                nc.sync.dma_start(
                    eg[:].rearrange("p a b -> p (a b)"),
                    egd[:, g * 8 * EGW : (g + 1) * 8 * EGW],
                )
                sg = sgp.tile([P, 8, P], dt.float16, tag="sg")
                nc.sync.dma_start(
                    sg[:].rearrange("p a b -> p (a b)"),
                    sgd[:, g * 8 * P : (g + 1) * 8 * P],
                )
                featg = fp.tile([P, 8, 64], dt.float16, tag="feat")

                for h in range(2):  # 512-edge halves
                    hsb = hpool.tile([P, 2, 512], dt.float16, tag="h")
                    for qh in range(2):  # FC1 per 256 edges (1 PSUM bank)
                        hpt = hps.tile([P, 2, 256], dt.float32, tag="hp")
                        rhs_sc = scT_sb[
                            :,
                            g * GG_E + h * 512 + qh * 256 : g * GG_E
                            + h * 512
                            + (qh + 1) * 256,
                        ]
                        for kc in range(2):
                            nc.tensor.matmul(
                                out=hpt[:, kc, :],
                                lhsT=fw1_sb[:, kc * 128 : (kc + 1) * 128],
                                rhs=rhs_sc,
                                start=True,
                                stop=True,
                            )
                        nc.scalar.activation(
                            hsb[:, :, qh * 256 : (qh + 1) * 256], hpt[:], Act.Relu
                        )

                    for sj in range(2):  # 256-edge supertiles within half
                        s = 2 * h + sj
                        w16 = wpool.tile([P, 2, 1024], dt.float16, tag="w16")
                        for jj in range(2):  # 128-edge subtiles
                            wpt = wps.tile([P, 1024], dt.float32, tag="wp")
                            for nh in range(2):
                                for kc in range(2):
                                    nc.tensor.matmul(
                                        out=wpt[:, nh * 512 : (nh + 1) * 512],
                                        lhsT=hsb[
                                            :, kc, (2 * sj + jj) * 128 : (2 * sj + jj + 1) * 128
                                        ],
                                        rhs=fw2_sb[
                                            :, kc * 1024 + nh * 512 : kc * 1024 + (nh + 1) * 512
                                        ],
                                        start=(kc == 0),
                                        stop=(kc == 1),
                                    )
                            nc.scalar.activation(w16[:, jj, :], wpt[:], Act.Copy)

                        # ---- tensor product on 256 edges ----
                        # w16 cols: [w12 (a,o,i) 512 | w3 (o,i) 256 | w4 (o,i) 256]
                        # prod cols: [pr12 512 | pr3 256 | pr4 (o,c,i) 768]
                        prod = dv.tile([P, 2, 1536], dt.float16, tag="prod")
                        for jj in range(2):
                            ej = eg[:, 2 * s + jj, :]
                            in1_12 = bass.AP(
                                ej.tensor,
                                ej.offset,
                                [list(ej.ap[0]), [64, 2], [0, 16], [1, 16]],
                            )
                            nc.vector.tensor_tensor(
                                out=prod[:, jj, 0:512].rearrange(
                                    "p (a o i) -> p a o i", a=2, i=16
                                ),
                                in0=w16[:, jj, 0:512].rearrange(
                                    "p (a o i) -> p a o i", a=2, i=16
                                ),
                                in1=in1_12,
                                op=Alu.mult,
                            )
                            # path 3 on GpSimd: w3 * s
                            nc.gpsimd.tensor_tensor(
                                out=prod[:, jj, 512:768].rearrange(
                                    "p (o i) -> p o i", i=16
                                ),
                                in0=w16[:, jj, 512:768].rearrange(
                                    "p (o i) -> p o i", i=16
                                ),
                                in1=eg[:, 2 * s + jj, 80:96]
                                .unsqueeze(1)
                                .broadcast_to([P, 16, 16]),
                                op=Alu.mult,
                            )
                            for cc in range(3):  # path 4 per component, (c,o,i)
                                nc.vector.tensor_tensor(
                                    out=prod[
                                        :, jj, 768 + 256 * cc : 1024 + 256 * cc
                                    ].rearrange("p (o i) -> p o i", i=16),
                                    in0=w16[:, jj, 768:1024].rearrange(
                                        "p (o i) -> p o i", i=16
                                    ),
                                    in1=eg[
                                        :, 2 * s + jj, 16 + 16 * cc : 32 + 16 * cc
                                    ]
                                    .unsqueeze(1)
                                    .broadcast_to([P, 16, 16]),
                                    op=Alu.mult,
                                )
                        # paths 1/2/3: full pairwise add tree [P, e, 48, 16] -> [P, e, 48]
                        prA = prod[:, :, 0:768].rearrange(
                            "p e (g i) -> p e g i", i=16
                        )
                        trA1 = dv.tile([P, 2, 384], dt.float16, tag="trA1")
                        rA1 = trA1[:].rearrange("p e (g i) -> p e g i", i=8)
                        nc.vector.tensor_tensor(
                            out=rA1, in0=prA[:, :, :, 0:8], in1=prA[:, :, :, 8:16], op=Alu.add
                        )
                        trA2 = dv.tile([P, 2, 192], dt.float16, tag="trA2")
                        rA2 = trA2[:].rearrange("p e (g i) -> p e g i", i=4)
                        nc.vector.tensor_tensor(
                            out=rA2, in0=rA1[:, :, :, 0:4], in1=rA1[:, :, :, 4:8], op=Alu.add
                        )
                        trA3 = dv.tile([P, 2, 96], dt.float16, tag="trA3")
                        rA3 = trA3[:].rearrange("p e (g i) -> p e g i", i=2)
                        nc.vector.tensor_tensor(
                            out=rA3, in0=rA2[:, :, :, 0:2], in1=rA2[:, :, :, 2:4], op=Alu.add
                        )
                        mfa = dv.tile([P, 2, 48], dt.float16, tag="mfa")
                        nc.vector.tensor_tensor(
                            out=mfa[:],
                            in0=rA3[:, :, :, 0:1].squeeze(3),
                            in1=rA3[:, :, :, 1:2].squeeze(3),
                            op=Alu.add,
                        )
                        # path 4: reduce 16 -> 8 only; scatter matmul finishes it
                        prB = prod[:, :, 768:1536].rearrange(
                            "p e (g i) -> p e g i", i=16
                        )
                        trB1 = dv.tile([P, 2, 384], dt.float16, tag="trB1")
                        rB1 = trB1[:].rearrange("p e (g i) -> p e g i", i=8)
                        nc.vector.tensor_tensor(
                            out=rB1, in0=prB[:, :, :, 0:8], in1=prB[:, :, :, 8:16], op=Alu.add
                        )
                        # mfa per edge: [M1 | M2 | M3]; features on GpSimd
                        nc.gpsimd.tensor_tensor(
                            out=featg[:, 2 * s : 2 * s + 2, 16:64].rearrange(
                                "p e (c o) -> p e c o", c=3
                            ),
                            in0=mfa[:, :, 32:48]
                            .unsqueeze(2)
                            .broadcast_to([P, 2, 3, 16]),
                            in1=eg[:, 2 * s : 2 * s + 2, 96:99]
                            .unsqueeze(3)
                            .broadcast_to([P, 2, 3, 16]),
                            op=Alu.mult,
                        )
                        nc.gpsimd.tensor_tensor(
                            out=featg[:, 2 * s : 2 * s + 2, 0:16],
                            in0=mfa[:, :, 0:16],
                            in1=mfa[:, :, 16:32],
                            op=Alu.add,
                        )

                        # ---- one-hot scatter into the block accumulators ----
                        for jj in range(2):
                            t = g * 8 + s * 2 + jj
                            b, q = t // t_b, t % t_b
                            if q == 0:
                                acc = aps.tile([P, 64], dt.float32, tag="acc")
                                acc4 = aps4.tile([P, 384], dt.float32, tag="acc4")
                            nc.tensor.matmul(
                                out=acc[:],
                                lhsT=sg[:, 2 * s + jj, :],
                                rhs=featg[:, 2 * s + jj, :],
                                start=(q == 0),
                                stop=(q == t_b - 1),
                            )
                            nc.tensor.matmul(
                                out=acc4[:],
                                lhsT=sg[:, 2 * s + jj, :],
                                rhs=trB1[:, jj, :],
                                start=(q == 0),
                                stop=(q == t_b - 1),
                            )
                            if q == t_b - 1:
                                osb = ob.tile([P, 64], dt.float32, tag="osb")
                                nc.scalar.activation(osb[:], acc[:], Act.Copy)
                                a4 = ob.tile([P, 384], dt.float16, tag="a4")
                                nc.scalar.activation(a4[:], acc4[:], Act.Copy)
                                e1 = ob.tile([P, 192], dt.float16, tag="e1")
                                a4r = a4[:].rearrange("p (g i) -> p g i", i=8)
                                nc.vector.tensor_tensor(
                                    out=e1[:].rearrange("p (g i) -> p g i", i=4),
                                    in0=a4r[:, :, 0:4],
                                    in1=a4r[:, :, 4:8],
                                    op=Alu.add,
                                )
                                e2 = ob.tile([P, 96], dt.float16, tag="e2")
                                e1r = e1[:].rearrange("p (g i) -> p g i", i=4)
                                nc.vector.tensor_tensor(
                                    out=e2[:].rearrange("p (g i) -> p g i", i=2),
                                    in0=e1r[:, :, 0:2],
                                    in1=e1r[:, :, 2:4],
                                    op=Alu.add,
                                )
                                e3 = ob.tile([P, 48], dt.float16, tag="e3")
                                e2r = e2[:].rearrange("p (g i) -> p g i", i=2)
                                nc.vector.tensor_tensor(
                                    out=e3[:],
                                    in0=e2r[:, :, 0:1].squeeze(2),
                                    in1=e2r[:, :, 1:2].squeeze(2),
                                    op=Alu.add,
                                )
                                osb2 = ob.tile([P, 64], dt.float32, tag="osb2")
                                nc.vector.tensor_copy(osb2[:, 0:16], osb[:, 0:16])
                                nc.vector.tensor_tensor(
                                    out=osb2[:, 16:64],
                                    in0=osb[:, 16:64],
                                    in1=e3[:],
                                    op=Alu.add,
                                )
                                nc.sync.dma_start(
                                    outp[b * BLK : (b + 1) * BLK, :], osb2[:]
                                )
    nc.compile()
    return nc


def _prep(inputs):
    nf = np.asarray(inputs["node_features"], dtype=np.float32)
    src = np.asarray(inputs["edge_src"]).astype(np.int64)
    dst = np.asarray(inputs["edge_dst"]).astype(np.int64)
    attr = np.asarray(inputs["edge_attr"], dtype=np.float32)
    sc = np.asarray(inputs["edge_scalars"], dtype=np.float32)
    w1 = np.asarray(inputs["fc_w1"], dtype=np.float32)
    w2 = np.asarray(inputs["fc_w2"], dtype=np.float32)

    fw1 = np.ascontiguousarray((w1 / np.sqrt(3.0)).astype(np.float16))
    w2r = w2.reshape(256, 4, MUL, MUL).transpose(0, 1, 3, 2).copy()
    scale = (
        (1.0 / np.sqrt(256.0))
        * (1.0 / np.sqrt(2.0 * MUL))
        * (1.0 / np.sqrt(16.0))
    )
    w2r *= scale
    w2r[:, 1] *= 1.0 / np.sqrt(3.0)  # dot normalization (path 2 only)
    w2f = w2r.reshape(256, 1024)  # [w1|w2 (512) | w3 (256) | w4 (256)]
    fw2d = np.ascontiguousarray(
        w2f.reshape(2, 128, 1024).transpose(1, 0, 2).reshape(128, 2048).astype(np.float16)
    )

    order = np.argsort(dst, kind="stable")
    srcs, dsts = src[order], dst[order]
    attrs, scs = attr[order], sc[order]

    core_of = dsts // NODES_PER_CORE
    local = dsts - core_of * NODES_PER_CORE
    blk = local // BLK
    gb = core_of * BLOCKS + blk
    counts = np.bincount(gb, minlength=N_CORES * BLOCKS)
    t_b = max(2, int(math.ceil(counts.max() / P)))
    if t_b % 2:
        t_b += 1  # supertile (256-edge) alignment
    n_tiles = BLOCKS * t_b
    EP = n_tiles * P

    seg = np.zeros(N_CORES * BLOCKS + 1, np.int64)
    np.cumsum(counts, out=seg[1:])

    v_ci = (
        nf[:, 16:64].reshape(N_NODES, 16, 3).transpose(0, 2, 1).reshape(N_NODES, 48)
    )  # (c,i)

    in_maps = []
    for c in range(N_CORES):
        s_c = np.zeros(EP, np.int64)
        d_c = np.full(EP, -1, np.int64)
        at = np.zeros((EP, 4), np.float32)
        scc = np.zeros((EP, 3), np.float32)
        for b in range(BLOCKS):
            gidx = c * BLOCKS + b
            a0, a1 = int(seg[gidx]), int(seg[gidx + 1])
            n = a1 - a0
            off = b * t_b * P
            s_c[off : off + n] = srcs[a0:a1]
            d_c[off : off + n] = local[a0:a1] - b * BLK
            at[off : off + n] = attrs[a0:a1]
            scc[off : off + n] = scs[a0:a1]

        # per-edge packed stream [EP, EGW]:
        # [shs*s(16) | shs*v_ci(48) | dsc(16) | s(16) | shv(3) | pad(1)]
        sE = nf[s_c, 0:16]  # [EP, 16]
        vE = v_ci[s_c]  # [EP, 48]
        shs = at[:, 0:1]
        shv = at[:, 1:4]
        dscE = (
            (vE.reshape(EP, 3, 16) * shv[:, :, None]).sum(axis=1)
        )  # [EP, 16]
        eg = np.zeros((EP, EGW), np.float32)
        eg[:, 0:16] = shs * sE
        eg[:, 16:64] = shs * vE
        eg[:, 64:80] = dscE
        eg[:, 80:96] = sE
        eg[:, 96:99] = shv
        egt = np.ascontiguousarray(
            eg.reshape(n_tiles, P, EGW).transpose(1, 0, 2).reshape(P, -1)
        ).astype(np.float16)

        # one-hot dst columns [EP, 128] fp16 (zero column for padding)
        S = np.zeros((EP, P), np.float16)
        valid = d_c >= 0
        S[np.nonzero(valid)[0], d_c[valid]] = 1.0
        Sgt = np.ascontiguousarray(
            S.reshape(n_tiles, P, P).transpose(1, 0, 2).reshape(P, -1)
        )

        in_maps.append(
            {
                "egd": egt,
                "sgd": Sgt,
                "scTd": np.ascontiguousarray(scc.T.astype(np.float16)),
                "fw1": fw1,
                "fw2d": fw2d,
            }
        )
    return in_maps, t_b, 0


def kernel(**inputs) -> np.ndarray:
    from concourse.bass_interp import get_hw_module
    from concourse.bass_utils import run_bass_kernel_spmd

    in_maps, t_b, z = _prep(inputs)
    key = (t_b, z)
    if key not in _CACHE:
        _CACHE[key] = _build(t_b, z)
    nc = _CACHE[key]
    old = nc.m
    nc.m = get_hw_module(nc.m)
    try:
        res = run_bass_kernel_spmd(nc, in_maps, core_ids=list(range(N_CORES)))
    finally:
        nc.m = old
    out = np.concatenate(
        [res.results[c]["outp"][:NODES_PER_CORE] for c in range(N_CORES)], axis=0
    ).astype(np.float32)
    # device vector part is (c,o); reference wants (o,c)
    out[:, 16:64] = (
        out[:, 16:64].reshape(-1, 3, MUL).transpose(0, 2, 1).reshape(-1, 48)
    )
    return np.ascontiguousarray(out)


# revision 49
# speedup vs baseline: 1.0624x; 1.0624x over previous
"""Trainium2 Bass kernel: e3nn-style GNN convolution (FC-net edge weights ->
FullyConnectedTensorProduct -> scatter-sum over edge_dst).

v3 strategy (edge-parallel, dst-sharded, host pre-gather):
  * Sort edges by dst on host. Core c owns dst nodes [2500c, 2500(c+1)).
  * Host precomputes per padded edge slot (20 blocks x t_b tiles x 128):
      eg  = [shs*s(16) | shs*v_ci(48) | dsc(16) | s(16) | shv(3) | pad] (100)
      Sg  = one-hot dst column (128, fp16)
      scT = edge scalars (3, for the on-device FC net)
  * Per 1024-edge group on device:
      - FC1 on PE per 512 edges (h^T in PSUM), relu on Act -> fp16
      - FC2 on PE per 128 edges: w[e,1024] in PSUM, Act copy -> fp16 SBUF
      - products: paths 1/2/4 on DVE (fp16 packed, 2x mode), path 3 and the
        final feature assembly on GpSimd
      - reduction over i as a pairwise tensor_tensor add tree on DVE
      - per 128-edge tile: one-hot dst matmul accumulates the block's
        [128,64] output in PSUM; block end -> Act copy -> DMA out.
  * Host concatenates the 8 node-sharded outputs.

All normalization constants are folded into fc_w1/fc_w2 on the host.
"""

import math

import numpy as np

N_NODES = 20000
N_CORES = 8
NODES_PER_CORE = N_NODES // N_CORES  # 2500
P = 128
MUL = 16
BLK = 128
BLOCKS = NODES_PER_CORE // BLK + 1  # 20 blocks cover 2560 rows
GG_E = 1024
OUT_ROWS = BLOCKS * BLK  # 2560
EGW = 100  # per-edge packed stream width

_CACHE: dict = {}


def _build(t_b: int, _unused: int = 0):
    import concourse.bass as bass
    import concourse.mybir as mybir
    import concourse.tile as tile
    from concourse import bacc

    dt = mybir.dt
    Alu = mybir.AluOpType
    Act = mybir.ActivationFunctionType

    n_tiles = BLOCKS * t_b
    EP = n_tiles * P
    assert EP % GG_E == 0
    ngg = EP // GG_E

    nc = bacc.Bacc("TRN2", target_bir_lowering=False, debug=False)

    egd = nc.dram_tensor("egd", [P, n_tiles * EGW], dt.float16, kind="ExternalInput")
    sgd = nc.dram_tensor("sgd", [P, n_tiles * P], dt.float16, kind="ExternalInput")
    scTd = nc.dram_tensor("scTd", [3, EP], dt.float16, kind="ExternalInput")
    fw1 = nc.dram_tensor("fw1", [3, 256], dt.float16, kind="ExternalInput")
    fw2d = nc.dram_tensor("fw2d", [P, 2048], dt.float16, kind="ExternalInput")
    outp = nc.dram_tensor("outp", [OUT_ROWS, 64], dt.float32, kind="ExternalOutput")

    with tile.TileContext(nc) as tc:
        with (
            tc.tile_pool(name="const", bufs=1) as cp,
            tc.tile_pool(name="egp", bufs=3) as egp,
            tc.tile_pool(name="sgp", bufs=3) as sgp,
            tc.tile_pool(name="fp", bufs=3) as fp,
            tc.tile_pool(name="hp", bufs=3) as hpool,
            tc.tile_pool(name="wp16", bufs=3) as wpool,
            tc.tile_pool(name="dv", bufs=2) as dv,
            tc.tile_pool(name="ob", bufs=2) as ob,
            tc.tile_pool(name="hps", bufs=1, space="PSUM") as hps,
            tc.tile_pool(name="wps", bufs=2, space="PSUM") as wps,
            tc.tile_pool(name="aps", bufs=1, space="PSUM") as aps,
            tc.tile_pool(name="aps4", bufs=1, space="PSUM") as aps4,
        ):
            scT_sb = cp.tile([3, EP], dt.float16)
            nc.sync.dma_start(scT_sb[:], scTd[:])
            fw1_sb = cp.tile([3, 256], dt.float16)
            nc.sync.dma_start(fw1_sb[:], fw1[:])
            fw2_sb = cp.tile([P, 2048], dt.float16)
            nc.sync.dma_start(fw2_sb[:], fw2d[:])

            acc = None
            for g in range(ngg):
                eg = egp.tile([P, 8, EGW], dt.float16, tag="eg")
                nc.sync.dma_start(
                    eg[:].rearrange("p a b -> p (a b)"),
                    egd[:, g * 8 * EGW : (g + 1) * 8 * EGW],
                )
                sg = sgp.tile([P, 8, P], dt.float16, tag="sg")
                nc.sync.dma_start(
                    sg[:].rearrange("p a b -> p (a b)"),
                    sgd[:, g * 8 * P : (g + 1) * 8 * P],
                )
                featg = fp.tile([P, 8, 64], dt.float16, tag="feat")

                for h in range(2):  # 512-edge halves
                    hsb = hpool.tile([P, 2, 512], dt.float16, tag="h")
                    for qh in range(2):  # FC1 per 256 edges (1 PSUM bank)
                        hpt = hps.tile([P, 2, 256], dt.float32, tag="hp")
                        rhs_sc = scT_sb[
                            :,
                            g * GG_E + h * 512 + qh * 256 : g * GG_E
                            + h * 512
                            + (qh + 1) * 256,
                        ]
                        for kc in range(2):
                            nc.tensor.matmul(
                                out=hpt[:, kc, :],
                                lhsT=fw1_sb[:, kc * 128 : (kc + 1) * 128],
                                rhs=rhs_sc,
                                start=True,
                                stop=True,
                            )
                        nc.scalar.activation(
                            hsb[:, :, qh * 256 : (qh + 1) * 256], hpt[:], Act.Relu
                        )

                    for sj in range(2):  # 256-edge supertiles within half
                        s = 2 * h + sj
                        w16 = wpool.tile([P, 2, 1024], dt.float16, tag="w16")
                        for jj in range(2):  # 128-edge subtiles
                            wpt = wps.tile([P, 1024], dt.float32, tag="wp")
                            for nh in range(2):
                                for kc in range(2):
                                    nc.tensor.matmul(
                                        out=wpt[:, nh * 512 : (nh + 1) * 512],
                                        lhsT=hsb[
                                            :, kc, (2 * sj + jj) * 128 : (2 * sj + jj + 1) * 128
                                        ],
                                        rhs=fw2_sb[
                                            :, kc * 1024 + nh * 512 : kc * 1024 + (nh + 1) * 512
                                        ],
                                        start=(kc == 0),
                                        stop=(kc == 1),
                                    )
                            nc.scalar.activation(w16[:, jj, :], wpt[:], Act.Copy)

                        # ---- tensor product on 256 edges ----
                        # w16 cols: [w12 (a,o,i) 512 | w3 (o,i) 256 | w4 (o,i) 256]
                        # prod cols: [pr12 512 | pr3 256 | pr4 (o,c,i) 768]
                        prod = dv.tile([P, 2, 1536], dt.float16, tag="prod")
                        for jj in range(2):
                            ej = eg[:, 2 * s + jj, :]
                            in1_12 = bass.AP(
                                ej.tensor,
                                ej.offset,
                                [list(ej.ap[0]), [64, 2], [0, 16], [1, 16]],
                            )
                            nc.vector.tensor_tensor(
                                out=prod[:, jj, 0:512].rearrange(
                                    "p (a o i) -> p a o i", a=2, i=16
                                ),
                                in0=w16[:, jj, 0:512].rearrange(
                                    "p (a o i) -> p a o i", a=2, i=16
                                ),
                                in1=in1_12,
                                op=Alu.mult,
                            )
                            # path 3 on GpSimd: w3 * s
                            nc.gpsimd.tensor_tensor(
                                out=prod[:, jj, 512:768].rearrange(
                                    "p (o i) -> p o i", i=16
                                ),
                                in0=w16[:, jj, 512:768].rearrange(
                                    "p (o i) -> p o i", i=16
                                ),
                                in1=eg[:, 2 * s + jj, 80:96]
                                .unsqueeze(1)
                                .broadcast_to([P, 16, 16]),
                                op=Alu.mult,
                            )
                            nc.vector.tensor_tensor(
                                out=prod[:, jj, 768:1536].rearrange(
                                    "p (o c i) -> p o c i", c=3, i=16
                                ),
                                in0=w16[:, jj, 768:1024]
                                .rearrange("p (o i) -> p o i", i=16)
                                .unsqueeze(2)
                                .broadcast_to([P, 16, 3, 16]),
                                in1=eg[:, 2 * s + jj, 16:64]
                                .rearrange("p (c i) -> p c i", i=16)
                                .unsqueeze(1)
                                .broadcast_to([P, 16, 3, 16]),
                                op=Alu.mult,
                            )
                        # paths 1/2/3: full pairwise add tree [P, e, 48, 16] -> [P, e, 48]
                        prA = prod[:, :, 0:768].rearrange(
                            "p e (g i) -> p e g i", i=16
                        )
                        trA1 = dv.tile([P, 2, 384], dt.float16, tag="trA1")
                        rA1 = trA1[:].rearrange("p e (g i) -> p e g i", i=8)
                        nc.vector.tensor_tensor(
                            out=rA1, in0=prA[:, :, :, 0:8], in1=prA[:, :, :, 8:16], op=Alu.add
                        )
                        trA2 = dv.tile([P, 2, 192], dt.float16, tag="trA2")
                        rA2 = trA2[:].rearrange("p e (g i) -> p e g i", i=4)
                        nc.vector.tensor_tensor(
                            out=rA2, in0=rA1[:, :, :, 0:4], in1=rA1[:, :, :, 4:8], op=Alu.add
                        )
                        trA3 = dv.tile([P, 2, 96], dt.float16, tag="trA3")
                        rA3 = trA3[:].rearrange("p e (g i) -> p e g i", i=2)
                        nc.vector.tensor_tensor(
                            out=rA3, in0=rA2[:, :, :, 0:2], in1=rA2[:, :, :, 2:4], op=Alu.add
                        )
                        mfa = dv.tile([P, 2, 48], dt.float16, tag="mfa")
                        nc.vector.tensor_tensor(
                            out=mfa[:],
                            in0=rA3[:, :, :, 0:1].squeeze(3),
                            in1=rA3[:, :, :, 1:2].squeeze(3),
                            op=Alu.add,
                        )
                        # path 4: reduce 16 -> 8 only; scatter matmul finishes it
                        prB = prod[:, :, 768:1536].rearrange(
                            "p e (g i) -> p e g i", i=16
                        )
                        trB1 = dv.tile([P, 2, 384], dt.float16, tag="trB1")
                        rB1 = trB1[:].rearrange("p e (g i) -> p e g i", i=8)
                        nc.vector.tensor_tensor(
                            out=rB1, in0=prB[:, :, :, 0:8], in1=prB[:, :, :, 8:16], op=Alu.add
                        )
                        # mfa per edge: [M1 | M2 | M3]; features on GpSimd
                        nc.gpsimd.tensor_tensor(
                            out=featg[:, 2 * s : 2 * s + 2, 16:64].rearrange(
                                "p e (o c) -> p e o c", c=3
                            ),
                            in0=mfa[:, :, 32:48]
                            .unsqueeze(3)
                            .broadcast_to([P, 2, 16, 3]),
                            in1=eg[:, 2 * s : 2 * s + 2, 96:99]
                            .unsqueeze(2)
                            .broadcast_to([P, 2, 16, 3]),
                            op=Alu.mult,
                        )
                        nc.gpsimd.tensor_tensor(
                            out=featg[:, 2 * s : 2 * s + 2, 0:16],
                            in0=mfa[:, :, 0:16],
                            in1=mfa[:, :, 16:32],
                            op=Alu.add,
                        )

                        # ---- one-hot scatter into the block accumulators ----
                        for jj in range(2):
                            t = g * 8 + s * 2 + jj
                            b, q = t // t_b, t % t_b
                            if q == 0:
                                acc = aps.tile([P, 64], dt.float32, tag="acc")
                                acc4 = aps4.tile([P, 384], dt.float32, tag="acc4")
                            nc.tensor.matmul(
                                out=acc[:],
                                lhsT=sg[:, 2 * s + jj, :],
                                rhs=featg[:, 2 * s + jj, :],
                                start=(q == 0),
                                stop=(q == t_b - 1),
                            )
                            nc.tensor.matmul(
                                out=acc4[:],
                                lhsT=sg[:, 2 * s + jj, :],
                                rhs=trB1[:, jj, :],
                                start=(q == 0),
                                stop=(q == t_b - 1),
                            )
                            if q == t_b - 1:
                                osb = ob.tile([P, 64], dt.float32, tag="osb")
                                nc.scalar.activation(osb[:], acc[:], Act.Copy)
                                a4 = ob.tile([P, 384], dt.float16, tag="a4")
                                nc.scalar.activation(a4[:], acc4[:], Act.Copy)
                                e1 = ob.tile([P, 192], dt.float16, tag="e1")
                                a4r = a4[:].rearrange("p (g i) -> p g i", i=8)
                                nc.vector.tensor_tensor(
                                    out=e1[:].rearrange("p (g i) -> p g i", i=4),
                                    in0=a4r[:, :, 0:4],
                                    in1=a4r[:, :, 4:8],
                                    op=Alu.add,
                                )
                                e2 = ob.tile([P, 96], dt.float16, tag="e2")
                                e1r = e1[:].rearrange("p (g i) -> p g i", i=4)
                                nc.vector.tensor_tensor(
                                    out=e2[:].rearrange("p (g i) -> p g i", i=2),
                                    in0=e1r[:, :, 0:2],
                                    in1=e1r[:, :, 2:4],
                                    op=Alu.add,
                                )
                                e3 = ob.tile([P, 48], dt.float16, tag="e3")
                                e2r = e2[:].rearrange("p (g i) -> p g i", i=2)
                                nc.vector.tensor_tensor(
                                    out=e3[:],
                                    in0=e2r[:, :, 0:1].squeeze(2),
                                    in1=e2r[:, :, 1:2].squeeze(2),
                                    op=Alu.add,
                                )
                                osb2 = ob.tile([P, 64], dt.float32, tag="osb2")
                                nc.vector.tensor_copy(osb2[:, 0:16], osb[:, 0:16])
                                nc.vector.tensor_tensor(
                                    out=osb2[:, 16:64],
                                    in0=osb[:, 16:64],
                                    in1=e3[:],
                                    op=Alu.add,
                                )
                                nc.sync.dma_start(
                                    outp[b * BLK : (b + 1) * BLK, :], osb2[:]
                                )
    nc.compile()
    return nc


def _prep(inputs):
    nf = np.asarray(inputs["node_features"], dtype=np.float32)
    src = np.asarray(inputs["edge_src"]).astype(np.int64)
    dst = np.asarray(inputs["edge_dst"]).astype(np.int64)
    attr = np.asarray(inputs["edge_attr"], dtype=np.float32)
    sc = np.asarray(inputs["edge_scalars"], dtype=np.float32)
    w1 = np.asarray(inputs["fc_w1"], dtype=np.float32)
    w2 = np.asarray(inputs["fc_w2"], dtype=np.float32)

    fw1 = np.ascontiguousarray((w1 / np.sqrt(3.0)).astype(np.float16))
    w2r = w2.reshape(256, 4, MUL, MUL).transpose(0, 1, 3, 2).copy()
    scale = (
        (1.0 / np.sqrt(256.0))
        * (1.0 / np.sqrt(2.0 * MUL))
        * (1.0 / np.sqrt(16.0))
    )
    w2r *= scale
    w2r[:, 1] *= 1.0 / np.sqrt(3.0)  # dot normalization (path 2 only)
    w2f = w2r.reshape(256, 1024)  # [w1|w2 (512) | w3 (256) | w4 (256)]
    fw2d = np.ascontiguousarray(
        w2f.reshape(2, 128, 1024).transpose(1, 0, 2).reshape(128, 2048).astype(np.float16)
    )

    order = np.argsort(dst, kind="stable")
    srcs, dsts = src[order], dst[order]
    attrs, scs = attr[order], sc[order]

    core_of = dsts // NODES_PER_CORE
    local = dsts - core_of * NODES_PER_CORE
    blk = local // BLK
    gb = core_of * BLOCKS + blk
    counts = np.bincount(gb, minlength=N_CORES * BLOCKS)
    t_b = max(2, int(math.ceil(counts.max() / P)))
    if t_b % 2:
        t_b += 1  # supertile (256-edge) alignment
    n_tiles = BLOCKS * t_b
    EP = n_tiles * P

    seg = np.zeros(N_CORES * BLOCKS + 1, np.int64)
    np.cumsum(counts, out=seg[1:])

    v_ci = (
        nf[:, 16:64].reshape(N_NODES, 16, 3).transpose(0, 2, 1).reshape(N_NODES, 48)
    )  # (c,i)

    in_maps = []
    for c in range(N_CORES):
        s_c = np.zeros(EP, np.int64)
        d_c = np.full(EP, -1, np.int64)
        at = np.zeros((EP, 4), np.float32)
        scc = np.zeros((EP, 3), np.float32)
        for b in range(BLOCKS):
            gidx = c * BLOCKS + b
            a0, a1 = int(seg[gidx]), int(seg[gidx + 1])
            n = a1 - a0
            off = b * t_b * P
            s_c[off : off + n] = srcs[a0:a1]
            d_c[off : off + n] = local[a0:a1] - b * BLK
            at[off : off + n] = attrs[a0:a1]
            scc[off : off + n] = scs[a0:a1]

        # per-edge packed stream [EP, EGW]:
        # [shs*s(16) | shs*v_ci(48) | dsc(16) | s(16) | shv(3) | pad(1)]
        sE = nf[s_c, 0:16]  # [EP, 16]
        vE = v_ci[s_c]  # [EP, 48]
        shs = at[:, 0:1]
        shv = at[:, 1:4]
        dscE = (
            (vE.reshape(EP, 3, 16) * shv[:, :, None]).sum(axis=1)
        )  # [EP, 16]
        eg = np.zeros((EP, EGW), np.float32)
        eg[:, 0:16] = shs * sE
        eg[:, 16:64] = shs * vE
        eg[:, 64:80] = dscE
        eg[:, 80:96] = sE
        eg[:, 96:99] = shv
        egt = np.ascontiguousarray(
            eg.reshape(n_tiles, P, EGW).transpose(1, 0, 2).reshape(P, -1)
        ).astype(np.float16)

        # one-hot dst columns [EP, 128] fp16 (zero column for padding)
        S = np.zeros((EP, P), np.float16)
        valid = d_c >= 0
        S[np.nonzero(valid)[0], d_c[valid]] = 1.0
        Sgt = np.ascontiguousarray(
            S.reshape(n_tiles, P, P).transpose(1, 0, 2).reshape(P, -1)
        )

        in_maps.append(
            {
                "egd": egt,
                "sgd": Sgt,
                "scTd": np.ascontiguousarray(scc.T.astype(np.float16)),
                "fw1": fw1,
                "fw2d": fw2d,
            }
        )
    return in_maps, t_b, 0


def kernel(**inputs) -> np.ndarray:
    from concourse.bass_interp import get_hw_module
    from concourse.bass_utils import run_bass_kernel_spmd

    in_maps, t_b, z = _prep(inputs)
    key = (t_b, z)
    if key not in _CACHE:
        _CACHE[key] = _build(t_b, z)
    nc = _CACHE[key]
    old = nc.m
    nc.m = get_hw_module(nc.m)
    try:
        res = run_bass_kernel_spmd(nc, in_maps, core_ids=list(range(N_CORES)))
    finally:
        nc.m = old
    out = np.concatenate(
        [res.results[c]["outp"][:NODES_PER_CORE] for c in range(N_CORES)], axis=0
    )
    return np.ascontiguousarray(out.astype(np.float32))
